# revision 15
# baseline (speedup 1.0000x reference)
"""Trainium2 Bass kernel for MultiHeadSelfAttention + residual + LayerNorm.

Problem: x[2, 2048, 1024], 16 heads, head_dim 64, fp32 I/O.
  Q/K/V = x @ W{q,k,v}.T + b;  attn = softmax(Q K^T / 8) V
  out = attn-concat @ Wo.T + bo;  y = LayerNorm(x + out)

Sharding (8 cores, collective-free):
  core c: batch b = c // 4, query-token strip q = c % 4 (512 tokens).
  Each core computes K/V for its whole batch (all 16 heads), Q for its
  512 query tokens, full attention + out-proj + LayerNorm for them, and
  outputs out[512, 1024].  K/V projection is recomputed 4x per batch --
  cheaper than the measured collective alternatives for this shape.

v2 design (PE was the bottleneck at ~259us busy of a 310us span):
  - ALL matmuls run fp8 e4m3 (x, Wq, Wk, Wv, Wo, K, Q, P, V, ctx in fp8)
    with DoubleRow pair-contraction for the projections and out-proj:
    halves the MM count of K/Q/out projections vs the bf16 baseline.
  - Weights are pre-scaled host-side to sit in the e4m3 normal range
    (Wq,Wk x8; Wv x16; Wo x32) and the residual input is pre-scaled x512
    so the out-proj PSUM lands at 512*(out+x).  LayerNorm is
    scale-invariant, so only eps is scaled (x512^2); gamma/beta epilogue
    is unaffected.  Score scale absorbs the 8*8: exp(scale=0.125/64).
  - Scores (contraction = head_dim 64 -> only half the PE rows) run as
    row-tiled CONCURRENT pairs: head-even weights in array rows 0-63,
    head-odd in rows 64-127 (tile_position auto-derived from the base
    partition), sharing the 128x128 array per key chunk.
  - x is loaded once (fp8, 2MB) and stays resident; the bf16 copy of x
    is gone entirely (halves input DMA).
  - softmax exp: most chunks on the Scalar engine (Exp LUT, fp8 out);
    every 4th chunk is computed on the Vector engine instead with a
    Schraudolph bit-trick: bits = round(a*logits + 55.54) as uint8,
    bitcast to e4m3 (DVE converts with round-to-nearest, saturating).
    Zero-bias constant so ACT and DVE chunks agree in expectation;
    softmax renormalizes the shared multiplicative bias away anyway
    (the ones-column in V gives rowsums of the SAME quantized P).
  - rowsum reciprocal runs directly on the PSUM rowsum row ([1,512]),
    then gpsimd partition-broadcasts the reciprocal (drops one DVE
    copy per head vs broadcasting the raw sum).
  - K/Q j-tiles j2..j7 and V quads 1..3 are emitted as PE filler inside
    the attention windows (two j-tiles ahead, half a quad per pair) so
    the PE never idles and the HAM clock stays warm.
Measured end-to-end Frobenius rel err ~1e-3 (tolerance 2e-2; errors in
the attention path are suppressed ~100x by the residual).
"""

import numpy as np
import ml_dtypes

P = 128
D = 1024
S = 2048
B = 2
H = 16
DH = 64
TQ = 512  # query tokens per core
N_CORES = 8

F32 = np.float32
BF16 = ml_dtypes.bfloat16
F8 = ml_dtypes.float8_e4m3fn

# host-side pre-scales (see docstring)
WQ_S = 8.0
WK_S = 8.0
WV_S = 16.0
WO_S = 32.0
RES_S = WV_S * WO_S  # 512
LN_EPS = 1e-5 * RES_S * RES_S
SC_SCALE = 0.125 / (WQ_S * WK_S)  # exp scale on raw psum logits
A_SCHR = SC_SCALE * 8.0 / 0.6931471805599453
B_SCHR = 55.54
SCHR = (5, 10, 15)  # chunk indices (of 16 per head-pair) done on DVE

_BUILT = {}

import os

KPHASE = int(os.environ.get("KPHASE", "3"))
KDEBUG = int(os.environ.get("KDEBUG", "0"))


def _build_nc(apply_gb=True):
    from contextlib import ExitStack

    import concourse.tile as tile
    from concourse import bacc, mybir

    bf = mybir.dt.bfloat16
    f8 = mybir.dt.float8e4
    u8 = mybir.dt.uint8
    f32 = mybir.dt.float32
    AX = mybir.AxisListType.X
    OP = mybir.AluOpType
    AF = mybir.ActivationFunctionType
    DR = mybir.MatmulPerfMode.DoubleRow

    nc = bacc.Bacc(
        "TRN2",
        target_bir_lowering=False,
        debug=False,
        enable_asserts=False,
        num_devices=N_CORES,
    )

    # ---- DRAM I/O ----
    x8_d = nc.dram_tensor("x8", [4, 8, P, 512], f8, kind="ExternalInput").ap()
    wq_d = nc.dram_tensor("wq", [D, D], f8, kind="ExternalInput").ap()
    wk_d = nc.dram_tensor("wk", [D, D], f8, kind="ExternalInput").ap()
    wv_d = nc.dram_tensor("wv", [D, D], f8, kind="ExternalInput").ap()
    wo_d = nc.dram_tensor("wo", [D, D], f8, kind="ExternalInput").ap()
    qb_d = nc.dram_tensor("qb", [P, 8], f32, kind="ExternalInput").ap()
    kb_d = nc.dram_tensor("kb", [P, 8], f32, kind="ExternalInput").ap()
    # rows: [16*bv | gamma | beta]
    rows_d = nc.dram_tensor("rows", [1, 3 * D], bf, kind="ExternalInput").ap()
    xres_d = nc.dram_tensor("xres", [TQ, D], f32, kind="ExternalInput").ap()
    out_d = nc.dram_tensor("out", [TQ, D], f32, kind="ExternalOutput").ap()
    if KDEBUG:
        dbg_k = nc.dram_tensor("dbg_k", [P, 8 * S], u8, kind="ExternalOutput").ap()
        dbg_q = nc.dram_tensor("dbg_q", [P, 8 * TQ], u8, kind="ExternalOutput").ap()
        dbg_v = nc.dram_tensor("dbg_v", [P, 16 * H * (DH + 1)], u8, kind="ExternalOutput").ap()
        dbg_c = nc.dram_tensor("dbg_c", [P, 8 * TQ], u8, kind="ExternalOutput").ap()
        dbg_pt = nc.dram_tensor("dbg_pt", [P, 8, 2, 2, 512], u8, kind="ExternalOutput").ap()
        dbg_cp = nc.dram_tensor("dbg_cp", [P, 2, 512], f32, kind="ExternalOutput").ap()

    wq_t = wq_d.rearrange("(o p) n -> p o n", p=P)  # [128, 8, 1024]
    wk_t = wk_d.rearrange("(o p) n -> p o n", p=P)
    wv_t = wv_d.rearrange("(o p) n -> p o n", p=P)
    wo_t = wo_d.rearrange("(o p) n -> p o n", p=P)

    with tile.TileContext(nc) as tc:
        with ExitStack() as ctx:
            # ---- pools ----
            consts = ctx.enter_context(tc.tile_pool(name="consts", bufs=1))
            wpool = ctx.enter_context(tc.tile_pool(name="wpool", bufs=1))
            big = ctx.enter_context(tc.tile_pool(name="big", bufs=1))
            ppool = ctx.enter_context(tc.tile_pool(name="ppool", bufs=3))
            spool = ctx.enter_context(tc.tile_pool(name="spool", bufs=4))
            hpool = ctx.enter_context(tc.tile_pool(name="hpool", bufs=3))
            xrpool = ctx.enter_context(tc.tile_pool(name="xrpool", bufs=4))
            pmm = ctx.enter_context(tc.tile_pool(name="pmm", bufs=2, space="PSUM"))
            smm = ctx.enter_context(tc.tile_pool(name="smm", bufs=2, space="PSUM"))
            ctxp = ctx.enter_context(tc.tile_pool(name="ctxp", bufs=2, space="PSUM"))

            # ---- constants ----
            zero_c = consts.tile([P, 1], f32, tag="zero_c")
            nc.vector.memset(zero_c[:], 0.0)
            nc.const_aps.aps[(f32, 0.0)] = zero_c[:]
            eps_c = consts.tile([P, 1], f32, tag="eps_c")
            nc.vector.memset(eps_c[:], LN_EPS)
            ones_l = consts.tile([1, P], bf, tag="ones_l")  # matmul lhsT ones
            nc.vector.memset(ones_l[:], 1.0)
            rows_sb = consts.tile([1, 3 * D], bf, tag="rows")
            nc.sync.dma_start(rows_sb[:], rows_d[:])
            qb_sb = consts.tile([P, 8], f32, tag="qb")
            nc.sync.dma_start(qb_sb[:], qb_d[:])
            kb_sb = consts.tile([P, 8], f32, tag="kb")
            nc.sync.dma_start(kb_sb[:], kb_d[:])

            # ---- resident inputs ----
            x8_sb = wpool.tile([P, 8, S], f8, tag="x8")
            wk8_sb = wpool.tile([P, 8, D], f8, tag="wk8")
            wv8_sb = wpool.tile([P, 8, D], f8, tag="wv8")
            wq8_sb = wpool.tile([P, 8, D], f8, tag="wq8")
            wo8_sb = wpool.tile([P, 8, D], f8, tag="wo8")
            for k in range(8):
                nc.sync.dma_start(x8_sb[:, k, 0:512], x8_d[0, k])
                nc.sync.dma_start(wk8_sb[:, k], wk_t[:, k])
            for s in range(1, 4):
                for k in range(8):
                    nc.sync.dma_start(x8_sb[:, k, s * 512 : (s + 1) * 512], x8_d[s, k])
            for k in range(8):
                nc.sync.dma_start(wq8_sb[:, k], wq_t[:, k])
            for k in range(8):
                nc.sync.dma_start(wv8_sb[:, k], wv_t[:, k])

            # broadcast [1, 1024] rows across partitions via rank-1 matmuls
            bv_bc = consts.tile([P, D], bf, tag="bv_bc")
            bcasts = [bv_bc]
            if apply_gb:
                ga_bc = consts.tile([P, D], bf, tag="ga_bc")
                be_bc = consts.tile([P, D], bf, tag="be_bc")
                bcasts += [ga_bc, be_bc]
            for idx, dst in enumerate(bcasts):
                for half in range(2):
                    ps = smm.tile([P, 2, 512], f32, tag="smm")
                    nc.tensor.matmul(
                        ps[:, 0],
                        ones_l[:],
                        rows_sb[:, idx * D + half * 512 : idx * D + (half + 1) * 512],
                        start=True,
                        stop=True,
                    )
                    nc.scalar.copy(dst[:, half * 512 : (half + 1) * 512], ps[:, 0])

            # ---- big activations ----
            kT8 = big.tile([P, 8, S], f8, tag="kT")  # K^T: [dh-pair part, j, token]
            qT8 = big.tile([P, 8, TQ], f8, tag="qT")
            # V' per (tk-chunk, head): [128 tok, 65] (64 dh + ones col)
            v_sb = big.tile([P, 16, H, DH + 1], f8, tag="v")
            nc.vector.memset(v_sb[:, :, :, DH : DH + 1], 1.0)
            ctxf = [
                big.tile([P, 2, TQ], f8, tag=f"ctxf{q}", name=f"ctxf{q}")
                for q in range(4)
            ]

            # ---- fp8 DoubleRow K/Q projection for one j-tile (all strips) ----
            def kq_proj_j(j):
                steps = []
                for s in range(4):
                    def kstep(s=s, j=j):
                        ps = pmm.tile([P, 512], f32, tag="pmm")
                        for c2 in range(4):
                            nc.tensor.matmul(
                                ps[:],
                                wk8_sb[:, 2 * c2 : 2 * c2 + 2, j * P : (j + 1) * P],
                                x8_sb[:, 2 * c2 : 2 * c2 + 2, s * 512 : (s + 1) * 512],
                                start=(c2 == 0),
                                stop=(c2 == 3),
                                perf_mode=DR,
                            )
                        nc.vector.tensor_scalar_add(
                            kT8[:, j, s * 512 : (s + 1) * 512], ps[:], kb_sb[:, j : j + 1]
                        )
                    steps.append(kstep)

                def qstep(j=j):
                    ps = pmm.tile([P, 512], f32, tag="pmm")
                    for c2 in range(4):
                        nc.tensor.matmul(
                            ps[:],
                            wq8_sb[:, 2 * c2 : 2 * c2 + 2, j * P : (j + 1) * P],
                            x8_sb[:, 2 * c2 : 2 * c2 + 2, 0:512],
                            start=(c2 == 0),
                            stop=(c2 == 3),
                            perf_mode=DR,
                        )
                    nc.vector.tensor_scalar_add(qT8[:, j], ps[:], qb_sb[:, j : j + 1])
                steps.append(qstep)
                return steps

            # ---- fp8 DoubleRow V projection steps for one quad ----
            def v_proj_quad(quad):
                steps = []
                for s in range(4):
                    for tc_ in range(4):
                        def vstep(s=s, tc_=tc_, quad=quad):
                            tchunk = s * 4 + tc_
                            ps = pmm.tile([P, 512], f32, tag="pmm")
                            for c2 in range(4):
                                nc.tensor.matmul(
                                    ps[:, : 4 * DH],
                                    x8_sb[:, 2 * c2 : 2 * c2 + 2, tchunk * P : (tchunk + 1) * P],
                                    wv8_sb[:, 2 * c2 : 2 * c2 + 2, quad * 256 : (quad + 1) * 256],
                                    start=(c2 == 0),
                                    stop=(c2 == 3),
                                    perf_mode=DR,
                                )
                            nc.vector.tensor_tensor(
                                v_sb[:, tchunk, quad * 4 : (quad + 1) * 4, 0:DH],
                                ps[:, : 4 * DH].rearrange("p (h d) -> p h d", d=DH),
                                bv_bc[:, quad * 256 : (quad + 1) * 256].rearrange(
                                    "p (h d) -> p h d", d=DH
                                ),
                                OP.add,
                            )
                        steps.append(vstep)
                return steps

            # upfront: j0, j1 K/Q projections + V quad 0
            for st in kq_proj_j(0):
                st()
            for st in kq_proj_j(1):
                st()
            for st in v_proj_quad(0):
                st()

            # filler schedule per attention pair (consumed one per chunk)
            def dma_fill():
                for k in range(8):
                    nc.sync.dma_start(wo8_sb[:, k], wo_t[:, k])

            xrs = []

            def xres_fill():
                for tt in range(4):
                    xr = xrpool.tile([P, D], f32, tag="xr", name=f"xr{tt}")
                    nc.sync.dma_start(xr[:], xres_d[tt * P : (tt + 1) * P, :])
                    xrs.append(xr)

            vq = [v_proj_quad(q) for q in (1, 2, 3)]
            fillers = [
                kq_proj_j(2) + vq[0][:8],
                kq_proj_j(3) + vq[0][8:],
                kq_proj_j(4) + vq[1][:8],
                kq_proj_j(5) + vq[1][8:],
                kq_proj_j(6) + vq[2][:8],
                kq_proj_j(7) + vq[2][8:],
                [dma_fill, xres_fill],
                [],
            ]

            # ---- attention: 8 head-pairs, row-tiled concurrent score MMs ----
            for pr in range(8):
                he, ho = 2 * pr, 2 * pr + 1
                fill = list(fillers[pr])
                if KPHASE < 2:
                    for st in fill:
                        st()
                    continue
                cps_e = ctxp.tile([P, 512], f32, tag="ctx")
                cps_o = ctxp.tile([P, 512], f32, tag="ctx")
                for cc in range(8):
                    pt = ppool.tile([P, 2, 2, 512], f8, tag="pt")
                    for par in range(2):
                        c = 2 * cc + par
                        sc = smm.tile([P, 2, 512], f32, tag="smm")
                        nc.tensor.matmul(
                            sc[:, 0],
                            kT8[0:DH, pr, c * P : (c + 1) * P],
                            qT8[0:DH, pr],
                            start=True,
                            stop=True,
                        )
                        nc.tensor.matmul(
                            sc[:, 1],
                            kT8[DH:P, pr, c * P : (c + 1) * P],
                            qT8[DH:P, pr],
                            start=True,
                            stop=True,
                        )
                        if c in SCHR:
                            nc.vector.tensor_scalar(
                                pt[:, par].bitcast(u8),
                                sc[:],
                                A_SCHR,
                                B_SCHR,
                                OP.mult,
                                OP.add,
                            )
                        else:
                            nc.scalar.activation(pt[:, par], sc[:], AF.Exp, scale=SC_SCALE)
                        if fill:
                            fill.pop(0)()
                    nc.tensor.matmul(
                        cps_e[: DH + 1],
                        v_sb[:, 2 * cc : 2 * cc + 2, he],
                        pt[:, :, 0],
                        start=(cc == 0),
                        stop=(cc == 7),
                        perf_mode=DR,
                    )
                    nc.tensor.matmul(
                        cps_o[: DH + 1],
                        v_sb[:, 2 * cc : 2 * cc + 2, ho],
                        pt[:, :, 1],
                        start=(cc == 0),
                        stop=(cc == 7),
                        perf_mode=DR,
                    )
                    if KDEBUG and pr == 0:
                        nc.sync.dma_start(dbg_pt[:, cc], pt[:].bitcast(u8))
                if KDEBUG and pr == 0:
                    cpcopy = hpool.tile([P, 2, 512], f32, tag="cpdbg")
                    nc.vector.tensor_copy(cpcopy[:, 0], cps_e[:])
                    nc.vector.tensor_copy(cpcopy[:, 1], cps_o[:])
                    nc.sync.dma_start(dbg_cp[:], cpcopy[:])
                for st in fill:
                    st()
                # evacuate ctx PSUM to SBUF right away (frees the bank for the
                # next pair), then normalize from the SBUF copy
                for h, cps in ((he, cps_e), (ho, cps_o)):
                    rs = spool.tile([1, 512], f32, tag="rs")
                    nc.vector.tensor_copy(rs[:], cps[DH : DH + 1, :])
                    cpy = spool.tile([DH, 512], f32, tag="cpy")
                    nc.vector.tensor_copy(cpy[:], cps[:DH, :])
                    ri = spool.tile([1, 512], f32, tag="ri")
                    nc.vector.reciprocal_approx_fast(ri[:], rs[:])
                    rb = spool.tile([DH, 512], f32, tag="rb")
                    nc.gpsimd.partition_broadcast(rb[:], ri[:])
                    po = (h % 2) * DH
                    nc.vector.tensor_tensor(
                        ctxf[h // 4][po : po + DH, (h % 4) // 2],
                        cpy[:],
                        rb[:],
                        OP.mult,
                    )

            # ---- out projection (fp8 DR) + residual + LayerNorm ----
            for tt in range(4):
                if KPHASE < 2:
                    continue
                xr = xrs[tt]
                if KPHASE < 3:
                    nc.sync.dma_start(out_d[tt * P : (tt + 1) * P, :], xr[:])
                    continue
                h_sb = hpool.tile([P, D], f32, tag="h")
                for half in range(2):
                    if half == 0:
                        ps = pmm.tile([P, 512], f32, tag="pmm")
                    else:
                        ps2 = smm.tile([P, 2, 512], f32, tag="smm")
                        ps = ps2[:, 0]
                    for j in range(4):
                        nc.tensor.matmul(
                            ps[:],
                            ctxf[j][:, :, tt * P : (tt + 1) * P],
                            wo8_sb[:, 2 * j : 2 * j + 2, half * 512 : (half + 1) * 512],
                            start=(j == 0),
                            stop=(j == 3),
                            perf_mode=DR,
                        )
                    # residual (+bo folded into xres host-side, x512 scale)
                    nc.vector.tensor_tensor(
                        h_sb[:, half * 512 : (half + 1) * 512],
                        ps[:],
                        xr[:, half * 512 : (half + 1) * 512],
                        OP.add,
                    )
                if KPHASE == 4:
                    nc.sync.dma_start(out_d[tt * P : (tt + 1) * P, :], h_sb[:])
                    continue
                # LayerNorm over the free dim (scale-invariant; eps pre-scaled)
                s1 = spool.tile([P, 1], f32, tag="s1")
                nc.vector.reduce_sum(s1[:], h_sb[:], axis=AX)
                y = hpool.tile([P, D], f32, tag="y")
                s2 = spool.tile([P, 1], f32, tag="s2")
                nc.scalar.activation(y[:], h_sb[:], AF.Square, accum_out=s2[:])
                mu = spool.tile([P, 1], f32, tag="mu")
                nc.scalar.mul(mu[:], s1[:], 1.0 / D)
                m2 = spool.tile([P, 1], f32, tag="m2")
                nc.scalar.square(m2[:], mu[:])
                var = spool.tile([P, 1], f32, tag="var")
                nc.vector.tensor_scalar(
                    var[:], s2[:], 1.0 / D, m2[:], OP.mult, OP.subtract
                )
                sd = spool.tile([P, 1], f32, tag="sd")
                nc.scalar.activation(sd[:], var[:], AF.Sqrt, bias=eps_c[:], scale=1.0)
                rstd = spool.tile([P, 1], f32, tag="rstd")
                nc.vector.reciprocal(rstd[:], sd[:])
                nc.vector.tensor_scalar(
                    y[:], h_sb[:], mu[:], rstd[:], OP.subtract, OP.mult
                )
                if apply_gb:
                    nc.vector.tensor_tensor(y[:], y[:], ga_bc[:], OP.mult)
                    nc.vector.tensor_tensor(y[:], y[:], be_bc[:], OP.add)
                nc.sync.dma_start(out_d[tt * P : (tt + 1) * P, :512], y[:, :512])
                nc.sync.dma_start(out_d[tt * P : (tt + 1) * P, 512:], y[:, 512:])

            if KDEBUG:
                nc.sync.dma_start(dbg_k[:], kT8[:].bitcast(u8).rearrange("p a b -> p (a b)"))
                nc.sync.dma_start(dbg_q[:], qT8[:].bitcast(u8).rearrange("p a b -> p (a b)"))
                nc.sync.dma_start(dbg_v[:], v_sb[:].bitcast(u8).rearrange("p a b c -> p (a b c)"))
                for q_ in range(4):
                    nc.sync.dma_start(
                        dbg_c[:, q_ * 2 * TQ : (q_ + 1) * 2 * TQ],
                        ctxf[q_][:].bitcast(u8).rearrange("p a b -> p (a b)"),
                    )

    nc.compile()
    return nc


def _get_nc(apply_gb=True):
    key = ("nc", apply_gb)
    if key not in _BUILT:
        _BUILT[key] = _build_nc(apply_gb)
    return _BUILT[key]


def _prep_in_maps(x, Wq, bq, Wk, bk, Wv, bv, Wo, bo, gamma, beta):
    x = np.asarray(x, F32)
    wq = np.ascontiguousarray(WQ_S * np.asarray(Wq, F32).T).astype(F8)
    wk = np.ascontiguousarray(WK_S * np.asarray(Wk, F32).T).astype(F8)
    wv = np.ascontiguousarray(WV_S * np.asarray(Wv, F32).T).astype(F8)
    wo = np.ascontiguousarray(WO_S * np.asarray(Wo, F32).T).astype(F8)
    qb = np.ascontiguousarray(WQ_S * np.asarray(bq, F32).reshape(8, P).T)
    kb = np.ascontiguousarray(WK_S * np.asarray(bk, F32).reshape(8, P).T)
    rows = (
        np.concatenate(
            [WV_S * np.asarray(bv, F32), np.asarray(gamma, F32), np.asarray(beta, F32)]
        )
        .reshape(1, 3 * D)
        .astype(BF16)
    )
    bo = np.asarray(bo, F32)
    xT = [np.ascontiguousarray(x[b].T) for b in range(B)]

    in_maps = []
    for c in range(N_CORES):
        b, q = c // 4, c % 4
        # permute: own query strip first; key order is irrelevant to attention
        perm = np.r_[q * TQ : (q + 1) * TQ, 0 : q * TQ, (q + 1) * TQ : S]
        in_maps.append(
            {
                "x8": np.ascontiguousarray(
                    xT[b][:, perm].reshape(8, P, 4, 512).transpose(2, 0, 1, 3)
                ).astype(F8),
                "wq": wq,
                "wk": wk,
                "wv": wv,
                "wo": wo,
                "qb": qb,
                "kb": kb,
                "rows": rows,
                "xres": RES_S
                * (np.ascontiguousarray(x[b, q * TQ : (q + 1) * TQ, :]) + bo[None, :]),
            }
        )
    return in_maps


def kernel(x, Wq, bq, Wk, bk, Wv, bv, Wo, bo, gamma, beta):
    from concourse.bass_utils import run_bass_kernel_spmd

    apply_gb = not (
        np.all(np.asarray(gamma, F32) == 1.0) and np.all(np.asarray(beta, F32) == 0.0)
    )
    nc = _get_nc(apply_gb)
    in_maps = _prep_in_maps(x, Wq, bq, Wk, bk, Wv, bv, Wo, bo, gamma, beta)
    res = run_bass_kernel_spmd(nc, in_maps, core_ids=list(range(N_CORES)))
    out = np.empty((B, S, D), F32)
    for c in range(N_CORES):
        b, q = c // 4, c % 4
        out[b, q * TQ : (q + 1) * TQ, :] = res.results[c]["out"]
    return out


# revision 18
# speedup vs baseline: 1.0459x; 1.0459x over previous
"""Trainium2 Bass kernel for MultiHeadSelfAttention + residual + LayerNorm.

Problem: x[2, 2048, 1024], 16 heads, head_dim 64, fp32 I/O.
  Q/K/V = x @ W{q,k,v}.T + b;  attn = softmax(Q K^T / 8) V
  out = attn-concat @ Wo.T + bo;  y = LayerNorm(x + out)

Sharding (8 cores, collective-free):
  core c: batch b = c // 4, query-token strip q = c % 4 (512 tokens).
  Each core computes K/V for its whole batch (all 16 heads), Q for its
  512 query tokens, full attention + out-proj + LayerNorm for them, and
  outputs out[512, 1024].  K/V projection is recomputed 4x per batch --
  cheaper than the measured collective alternatives for this shape.

v2 design (PE was the bottleneck at ~259us busy of a 310us span):
  - ALL matmuls run fp8 e4m3 (x, Wq, Wk, Wv, Wo, K, Q, P, V, ctx in fp8)
    with DoubleRow pair-contraction for the projections and out-proj:
    halves the MM count of K/Q/out projections vs the bf16 baseline.
  - Weights are pre-scaled host-side to sit in the e4m3 normal range
    (Wq,Wk x8; Wv x16; Wo x32) and the residual input is pre-scaled x512
    so the out-proj PSUM lands at 512*(out+x).  LayerNorm is
    scale-invariant, so only eps is scaled (x512^2); gamma/beta epilogue
    is unaffected.  Score scale absorbs the 8*8: exp(scale=0.125/64).
  - Scores (contraction = head_dim 64 -> only half the PE rows) run as
    row-tiled CONCURRENT pairs: head-even weights in array rows 0-63,
    head-odd in rows 64-127 (tile_position auto-derived from the base
    partition), sharing the 128x128 array per key chunk.
  - x is loaded once (fp8, 2MB) and stays resident; the bf16 copy of x
    is gone entirely (halves input DMA).
  - softmax exp: most chunks on the Scalar engine (Exp LUT, fp8 out);
    every 4th chunk is computed on the Vector engine instead with a
    Schraudolph bit-trick: bits = round(a*logits + 55.54) as uint8,
    bitcast to e4m3 (DVE converts with round-to-nearest, saturating).
    Zero-bias constant so ACT and DVE chunks agree in expectation;
    softmax renormalizes the shared multiplicative bias away anyway
    (the ones-column in V gives rowsums of the SAME quantized P).
  - rowsum reciprocal runs directly on the PSUM rowsum row ([1,512]),
    then gpsimd partition-broadcasts the reciprocal (drops one DVE
    copy per head vs broadcasting the raw sum).
  - K/Q j-tiles j2..j7 and V quads 1..3 are emitted as PE filler inside
    the attention windows (two j-tiles ahead, half a quad per pair) so
    the PE never idles and the HAM clock stays warm.
Measured end-to-end Frobenius rel err ~1e-3 (tolerance 2e-2; errors in
the attention path are suppressed ~100x by the residual).
"""

import numpy as np
import ml_dtypes

P = 128
D = 1024
S = 2048
B = 2
H = 16
DH = 64
TQ = 512  # query tokens per core
N_CORES = 8

F32 = np.float32
BF16 = ml_dtypes.bfloat16
F8 = ml_dtypes.float8_e4m3fn

# host-side pre-scales (see docstring)
WQ_S = 8.0
WK_S = 8.0
WV_S = 16.0
WO_S = 32.0
RES_S = WV_S * WO_S  # 512
LN_EPS = 1e-5 * RES_S * RES_S
SC_SCALE = 0.125 / (WQ_S * WK_S)  # exp scale on raw psum logits
A_SCHR = SC_SCALE * 8.0 / 0.6931471805599453
B_SCHR = 55.54
SCHR = (2, 6, 10, 14)  # chunk indices (of 16 per head-pair) done on DVE

_BUILT = {}

import os

KPHASE = int(os.environ.get("KPHASE", "3"))
KDEBUG = int(os.environ.get("KDEBUG", "0"))


def _build_nc(apply_gb=True):
    from contextlib import ExitStack

    import concourse.tile as tile
    from concourse import bacc, mybir

    bf = mybir.dt.bfloat16
    f8 = mybir.dt.float8e4
    u8 = mybir.dt.uint8
    f32 = mybir.dt.float32
    AX = mybir.AxisListType.X
    OP = mybir.AluOpType
    AF = mybir.ActivationFunctionType
    DR = mybir.MatmulPerfMode.DoubleRow

    nc = bacc.Bacc(
        "TRN2",
        target_bir_lowering=False,
        debug=False,
        enable_asserts=False,
        num_devices=N_CORES,
    )

    # ---- DRAM I/O ----
    x8_d = nc.dram_tensor("x8", [4, 8, P, 512], f8, kind="ExternalInput").ap()
    wq_d = nc.dram_tensor("wq", [D, D], f8, kind="ExternalInput").ap()
    wk_d = nc.dram_tensor("wk", [D, D], f8, kind="ExternalInput").ap()
    wv_d = nc.dram_tensor("wv", [D, D], f8, kind="ExternalInput").ap()
    wo_d = nc.dram_tensor("wo", [D, D], f8, kind="ExternalInput").ap()
    qb_d = nc.dram_tensor("qb", [P, 8], f32, kind="ExternalInput").ap()
    kb_d = nc.dram_tensor("kb", [P, 8], f32, kind="ExternalInput").ap()
    # rows: [16*bv | gamma | beta]
    rows_d = nc.dram_tensor("rows", [1, 3 * D], bf, kind="ExternalInput").ap()
    xres_d = nc.dram_tensor("xres", [TQ, D], f32, kind="ExternalInput").ap()
    out_d = nc.dram_tensor("out", [TQ, D], f32, kind="ExternalOutput").ap()
    if KDEBUG:
        dbg_k = nc.dram_tensor("dbg_k", [P, 8 * S], u8, kind="ExternalOutput").ap()
        dbg_q = nc.dram_tensor("dbg_q", [P, 8 * TQ], u8, kind="ExternalOutput").ap()
        dbg_v = nc.dram_tensor("dbg_v", [P, 16 * H * (DH + 1)], u8, kind="ExternalOutput").ap()
        dbg_c = nc.dram_tensor("dbg_c", [P, 8 * TQ], u8, kind="ExternalOutput").ap()
        dbg_pt = nc.dram_tensor("dbg_pt", [P, 8, 2, 2, 512], u8, kind="ExternalOutput").ap()
        dbg_cp = nc.dram_tensor("dbg_cp", [P, 2, 512], f32, kind="ExternalOutput").ap()

    wq_t = wq_d.rearrange("(o p) n -> p o n", p=P)  # [128, 8, 1024]
    wk_t = wk_d.rearrange("(o p) n -> p o n", p=P)
    wv_t = wv_d.rearrange("(o p) n -> p o n", p=P)
    wo_t = wo_d.rearrange("(o p) n -> p o n", p=P)

    with tile.TileContext(nc) as tc:
        with ExitStack() as ctx:
            # ---- pools ----
            consts = ctx.enter_context(tc.tile_pool(name="consts", bufs=1))
            wpool = ctx.enter_context(tc.tile_pool(name="wpool", bufs=1))
            big = ctx.enter_context(tc.tile_pool(name="big", bufs=1))
            ppool = ctx.enter_context(tc.tile_pool(name="ppool", bufs=3))
            spool = ctx.enter_context(tc.tile_pool(name="spool", bufs=4))
            hpool = ctx.enter_context(tc.tile_pool(name="hpool", bufs=3))
            xrpool = ctx.enter_context(tc.tile_pool(name="xrpool", bufs=4))
            pmm = ctx.enter_context(tc.tile_pool(name="pmm", bufs=2, space="PSUM"))
            smm = ctx.enter_context(tc.tile_pool(name="smm", bufs=2, space="PSUM"))
            ctxp = ctx.enter_context(tc.tile_pool(name="ctxp", bufs=2, space="PSUM"))

            # ---- constants ----
            zero_c = consts.tile([P, 1], f32, tag="zero_c")
            nc.vector.memset(zero_c[:], 0.0)
            nc.const_aps.aps[(f32, 0.0)] = zero_c[:]
            eps_c = consts.tile([P, 1], f32, tag="eps_c")
            nc.vector.memset(eps_c[:], LN_EPS)
            ones_l = consts.tile([1, P], bf, tag="ones_l")  # matmul lhsT ones
            nc.vector.memset(ones_l[:], 1.0)
            rows_sb = consts.tile([1, 3 * D], bf, tag="rows")
            nc.sync.dma_start(rows_sb[:], rows_d[:])
            qb_sb = consts.tile([P, 8], f32, tag="qb")
            nc.sync.dma_start(qb_sb[:], qb_d[:])
            kb_sb = consts.tile([P, 8], f32, tag="kb")
            nc.sync.dma_start(kb_sb[:], kb_d[:])

            # ---- resident inputs ----
            x8_sb = wpool.tile([P, 8, S], f8, tag="x8")
            wk8_sb = wpool.tile([P, 8, D], f8, tag="wk8")
            wv8_sb = wpool.tile([P, 8, D], f8, tag="wv8")
            wq8_sb = wpool.tile([P, 8, D], f8, tag="wq8")
            wo8_sb = wpool.tile([P, 8, D], f8, tag="wo8")
            for k in range(8):
                nc.sync.dma_start(x8_sb[:, k, 0:512], x8_d[0, k])
                nc.sync.dma_start(wk8_sb[:, k], wk_t[:, k])
            for k in range(8):
                nc.sync.dma_start(x8_sb[:, k, 512:1024], x8_d[1, k])
            for k in range(8):
                nc.sync.dma_start(wq8_sb[:, k], wq_t[:, k])
            for s in range(2, 4):
                for k in range(8):
                    nc.sync.dma_start(x8_sb[:, k, s * 512 : (s + 1) * 512], x8_d[s, k])
            for k in range(8):
                nc.sync.dma_start(wv8_sb[:, k], wv_t[:, k])

            # broadcast [1, 1024] rows across partitions via rank-1 matmuls
            bv_bc = consts.tile([P, D], bf, tag="bv_bc")
            bcasts = [bv_bc]
            if apply_gb:
                ga_bc = consts.tile([P, D], bf, tag="ga_bc")
                be_bc = consts.tile([P, D], bf, tag="be_bc")
                bcasts += [ga_bc, be_bc]
            for idx, dst in enumerate(bcasts):
                for half in range(2):
                    ps = smm.tile([P, 2, 512], f32, tag="smm")
                    nc.tensor.matmul(
                        ps[:, 0],
                        ones_l[:],
                        rows_sb[:, idx * D + half * 512 : idx * D + (half + 1) * 512],
                        start=True,
                        stop=True,
                    )
                    nc.scalar.copy(dst[:, half * 512 : (half + 1) * 512], ps[:, 0])

            # ---- big activations ----
            kT8 = big.tile([P, 8, S], f8, tag="kT")  # K^T: [dh-pair part, j, token]
            qT8 = big.tile([P, 8, TQ], f8, tag="qT")
            # V' per (tk-chunk, head): [128 tok, 65] (64 dh + ones col)
            v_sb = big.tile([P, 16, H, DH + 1], f8, tag="v")
            nc.vector.memset(v_sb[:, :, :, DH : DH + 1], 1.0)
            ctxf = [
                big.tile([P, 2, TQ], f8, tag=f"ctxf{q}", name=f"ctxf{q}")
                for q in range(4)
            ]

            # ---- fp8 DoubleRow K/Q projection for one j-tile (all strips) ----
            def kq_proj_j(j):
                steps = []
                for s in range(4):
                    def kstep(s=s, j=j):
                        ps = pmm.tile([P, 512], f32, tag="pmm")
                        for c2 in range(4):
                            nc.tensor.matmul(
                                ps[:],
                                wk8_sb[:, 2 * c2 : 2 * c2 + 2, j * P : (j + 1) * P],
                                x8_sb[:, 2 * c2 : 2 * c2 + 2, s * 512 : (s + 1) * 512],
                                start=(c2 == 0),
                                stop=(c2 == 3),
                                perf_mode=DR,
                            )
                        nc.vector.tensor_scalar_add(
                            kT8[:, j, s * 512 : (s + 1) * 512], ps[:], kb_sb[:, j : j + 1]
                        )
                    steps.append(kstep)

                def qstep(j=j):
                    ps = pmm.tile([P, 512], f32, tag="pmm")
                    for c2 in range(4):
                        nc.tensor.matmul(
                            ps[:],
                            wq8_sb[:, 2 * c2 : 2 * c2 + 2, j * P : (j + 1) * P],
                            x8_sb[:, 2 * c2 : 2 * c2 + 2, 0:512],
                            start=(c2 == 0),
                            stop=(c2 == 3),
                            perf_mode=DR,
                        )
                    nc.vector.tensor_scalar_add(qT8[:, j], ps[:], qb_sb[:, j : j + 1])
                steps.append(qstep)
                return steps

            # ---- fp8 DoubleRow V projection steps for one quad ----
            def v_proj_quad(quad):
                steps = []
                for s in range(4):
                    for tc_ in range(4):
                        def vstep(s=s, tc_=tc_, quad=quad):
                            tchunk = s * 4 + tc_
                            ps = pmm.tile([P, 512], f32, tag="pmm")
                            for c2 in range(4):
                                nc.tensor.matmul(
                                    ps[:, : 4 * DH],
                                    x8_sb[:, 2 * c2 : 2 * c2 + 2, tchunk * P : (tchunk + 1) * P],
                                    wv8_sb[:, 2 * c2 : 2 * c2 + 2, quad * 256 : (quad + 1) * 256],
                                    start=(c2 == 0),
                                    stop=(c2 == 3),
                                    perf_mode=DR,
                                )
                            nc.vector.tensor_tensor(
                                v_sb[:, tchunk, quad * 4 : (quad + 1) * 4, 0:DH],
                                ps[:, : 4 * DH].rearrange("p (h d) -> p h d", d=DH),
                                bv_bc[:, quad * 256 : (quad + 1) * 256].rearrange(
                                    "p (h d) -> p h d", d=DH
                                ),
                                OP.add,
                            )
                        steps.append(vstep)
                return steps

            # upfront: j0, j1 K/Q projections + V quad 0
            for st in kq_proj_j(0):
                st()
            for st in kq_proj_j(1):
                st()
            for st in v_proj_quad(0):
                st()

            # filler schedule per attention pair (consumed one per chunk)
            def dma_fill():
                for k in range(8):
                    nc.sync.dma_start(wo8_sb[:, k], wo_t[:, k])

            xrs = []

            def xres_fill():
                for tt in range(4):
                    xr = xrpool.tile([P, D], f32, tag="xr", name=f"xr{tt}")
                    nc.sync.dma_start(xr[:], xres_d[tt * P : (tt + 1) * P, :])
                    xrs.append(xr)

            vq = [v_proj_quad(q) for q in (1, 2, 3)]
            fillers = [
                kq_proj_j(2) + vq[0][:8],
                kq_proj_j(3) + vq[0][8:],
                kq_proj_j(4) + vq[1][:8],
                kq_proj_j(5) + vq[1][8:],
                kq_proj_j(6) + vq[2][:8],
                kq_proj_j(7) + vq[2][8:],
                [dma_fill, xres_fill],
                [],
            ]

            # ---- attention: 8 head-pairs, row-tiled concurrent score MMs ----
            for pr in range(8):
                he, ho = 2 * pr, 2 * pr + 1
                fill = list(fillers[pr])
                if KPHASE < 2:
                    for st in fill:
                        st()
                    continue
                cps_e = ctxp.tile([P, 512], f32, tag="ctx")
                cps_o = ctxp.tile([P, 512], f32, tag="ctx")
                for cc in range(8):
                    pt = ppool.tile([P, 2, 2, 512], f8, tag="pt")
                    for par in range(2):
                        c = 2 * cc + par
                        sc = smm.tile([P, 2, 512], f32, tag="smm")
                        nc.tensor.matmul(
                            sc[:, 0],
                            kT8[0:DH, pr, c * P : (c + 1) * P],
                            qT8[0:DH, pr],
                            start=True,
                            stop=True,
                        )
                        nc.tensor.matmul(
                            sc[:, 1],
                            kT8[DH:P, pr, c * P : (c + 1) * P],
                            qT8[DH:P, pr],
                            start=True,
                            stop=True,
                        )
                        if c in SCHR:
                            nc.vector.tensor_scalar(
                                pt[:, par].bitcast(u8),
                                sc[:],
                                A_SCHR,
                                B_SCHR,
                                OP.mult,
                                OP.add,
                            )
                        else:
                            nc.scalar.activation(pt[:, par], sc[:], AF.Exp, scale=SC_SCALE)
                        if fill:
                            fill.pop(0)()
                    nc.tensor.matmul(
                        cps_e[: DH + 1],
                        v_sb[:, 2 * cc : 2 * cc + 2, he],
                        pt[:, :, 0],
                        start=(cc == 0),
                        stop=(cc == 7),
                        perf_mode=DR,
                    )
                    nc.tensor.matmul(
                        cps_o[: DH + 1],
                        v_sb[:, 2 * cc : 2 * cc + 2, ho],
                        pt[:, :, 1],
                        start=(cc == 0),
                        stop=(cc == 7),
                        perf_mode=DR,
                    )
                    if KDEBUG and pr == 0:
                        nc.sync.dma_start(dbg_pt[:, cc], pt[:].bitcast(u8))
                if KDEBUG and pr == 0:
                    cpcopy = hpool.tile([P, 2, 512], f32, tag="cpdbg")
                    nc.vector.tensor_copy(cpcopy[:, 0], cps_e[:])
                    nc.vector.tensor_copy(cpcopy[:, 1], cps_o[:])
                    nc.sync.dma_start(dbg_cp[:], cpcopy[:])
                for st in fill:
                    st()
                # evacuate ctx PSUM to SBUF right away (frees the bank for the
                # next pair), then normalize from the SBUF copy
                for h, cps in ((he, cps_e), (ho, cps_o)):
                    rs = spool.tile([1, 512], f32, tag="rs")
                    nc.vector.tensor_copy(rs[:], cps[DH : DH + 1, :])
                    ri = spool.tile([1, 512], f32, tag="ri")
                    nc.vector.reciprocal_approx_fast(ri[:], rs[:])
                    rb = spool.tile([DH, 512], f32, tag="rb")
                    nc.gpsimd.partition_broadcast(rb[:], ri[:])
                    po = (h % 2) * DH
                    nc.vector.tensor_tensor(
                        ctxf[h // 4][po : po + DH, (h % 4) // 2],
                        cps[:DH],
                        rb[:],
                        OP.mult,
                    )

            # ---- out projection (fp8 DR) + residual + LayerNorm ----
            for tt in range(4):
                if KPHASE < 2:
                    continue
                xr = xrs[tt]
                if KPHASE < 3:
                    nc.sync.dma_start(out_d[tt * P : (tt + 1) * P, :], xr[:])
                    continue
                h_sb = hpool.tile([P, D], f32, tag="h")
                for half in range(2):
                    if half == 0:
                        ps = pmm.tile([P, 512], f32, tag="pmm")
                    else:
                        ps2 = smm.tile([P, 2, 512], f32, tag="smm")
                        ps = ps2[:, 0]
                    for j in range(4):
                        nc.tensor.matmul(
                            ps[:],
                            ctxf[j][:, :, tt * P : (tt + 1) * P],
                            wo8_sb[:, 2 * j : 2 * j + 2, half * 512 : (half + 1) * 512],
                            start=(j == 0),
                            stop=(j == 3),
                            perf_mode=DR,
                        )
                    # residual (+bo folded into xres host-side, x512 scale)
                    nc.vector.tensor_tensor(
                        h_sb[:, half * 512 : (half + 1) * 512],
                        ps[:],
                        xr[:, half * 512 : (half + 1) * 512],
                        OP.add,
                    )
                if KPHASE == 4:
                    nc.sync.dma_start(out_d[tt * P : (tt + 1) * P, :], h_sb[:])
                    continue
                # LayerNorm over the free dim (scale-invariant; eps pre-scaled)
                s1 = spool.tile([P, 1], f32, tag="s1")
                nc.vector.reduce_sum(s1[:], h_sb[:], axis=AX)
                y = hpool.tile([P, D], f32, tag="y")
                s2 = spool.tile([P, 1], f32, tag="s2")
                nc.scalar.activation(y[:], h_sb[:], AF.Square, accum_out=s2[:])
                mu = spool.tile([P, 1], f32, tag="mu")
                nc.scalar.mul(mu[:], s1[:], 1.0 / D)
                m2 = spool.tile([P, 1], f32, tag="m2")
                nc.scalar.square(m2[:], mu[:])
                var = spool.tile([P, 1], f32, tag="var")
                nc.vector.tensor_scalar(
                    var[:], s2[:], 1.0 / D, m2[:], OP.mult, OP.subtract
                )
                sd = spool.tile([P, 1], f32, tag="sd")
                nc.scalar.activation(sd[:], var[:], AF.Sqrt, bias=eps_c[:], scale=1.0)
                rstd = spool.tile([P, 1], f32, tag="rstd")
                nc.vector.reciprocal(rstd[:], sd[:])
                nc.vector.tensor_scalar(
                    y[:], h_sb[:], mu[:], rstd[:], OP.subtract, OP.mult
                )
                if apply_gb:
                    nc.vector.tensor_tensor(y[:], y[:], ga_bc[:], OP.mult)
                    nc.vector.tensor_tensor(y[:], y[:], be_bc[:], OP.add)
                nc.sync.dma_start(out_d[tt * P : (tt + 1) * P, :512], y[:, :512])
                nc.sync.dma_start(out_d[tt * P : (tt + 1) * P, 512:], y[:, 512:])

            if KDEBUG:
                nc.sync.dma_start(dbg_k[:], kT8[:].bitcast(u8).rearrange("p a b -> p (a b)"))
                nc.sync.dma_start(dbg_q[:], qT8[:].bitcast(u8).rearrange("p a b -> p (a b)"))
                nc.sync.dma_start(dbg_v[:], v_sb[:].bitcast(u8).rearrange("p a b c -> p (a b c)"))
                for q_ in range(4):
                    nc.sync.dma_start(
                        dbg_c[:, q_ * 2 * TQ : (q_ + 1) * 2 * TQ],
                        ctxf[q_][:].bitcast(u8).rearrange("p a b -> p (a b)"),
                    )

    nc.compile()
    return nc


def _get_nc(apply_gb=True):
    key = ("nc", apply_gb)
    if key not in _BUILT:
        _BUILT[key] = _build_nc(apply_gb)
    return _BUILT[key]


def _prep_in_maps(x, Wq, bq, Wk, bk, Wv, bv, Wo, bo, gamma, beta):
    x = np.asarray(x, F32)
    wq = np.ascontiguousarray(WQ_S * np.asarray(Wq, F32).T).astype(F8)
    wk = np.ascontiguousarray(WK_S * np.asarray(Wk, F32).T).astype(F8)
    wv = np.ascontiguousarray(WV_S * np.asarray(Wv, F32).T).astype(F8)
    wo = np.ascontiguousarray(WO_S * np.asarray(Wo, F32).T).astype(F8)
    qb = np.ascontiguousarray(WQ_S * np.asarray(bq, F32).reshape(8, P).T)
    kb = np.ascontiguousarray(WK_S * np.asarray(bk, F32).reshape(8, P).T)
    rows = (
        np.concatenate(
            [WV_S * np.asarray(bv, F32), np.asarray(gamma, F32), np.asarray(beta, F32)]
        )
        .reshape(1, 3 * D)
        .astype(BF16)
    )
    bo = np.asarray(bo, F32)
    xT = [np.ascontiguousarray(x[b].T) for b in range(B)]

    in_maps = []
    for c in range(N_CORES):
        b, q = c // 4, c % 4
        # permute: own query strip first; key order is irrelevant to attention
        perm = np.r_[q * TQ : (q + 1) * TQ, 0 : q * TQ, (q + 1) * TQ : S]
        in_maps.append(
            {
                "x8": np.ascontiguousarray(
                    xT[b][:, perm].reshape(8, P, 4, 512).transpose(2, 0, 1, 3)
                ).astype(F8),
                "wq": wq,
                "wk": wk,
                "wv": wv,
                "wo": wo,
                "qb": qb,
                "kb": kb,
                "rows": rows,
                "xres": RES_S
                * (np.ascontiguousarray(x[b, q * TQ : (q + 1) * TQ, :]) + bo[None, :]),
            }
        )
    return in_maps


def kernel(x, Wq, bq, Wk, bk, Wv, bv, Wo, bo, gamma, beta):
    from concourse.bass_utils import run_bass_kernel_spmd

    apply_gb = not (
        np.all(np.asarray(gamma, F32) == 1.0) and np.all(np.asarray(beta, F32) == 0.0)
    )
    nc = _get_nc(apply_gb)
    in_maps = _prep_in_maps(x, Wq, bq, Wk, bk, Wv, bv, Wo, bo, gamma, beta)
    res = run_bass_kernel_spmd(nc, in_maps, core_ids=list(range(N_CORES)))
    out = np.empty((B, S, D), F32)
    for c in range(N_CORES):
        b, q = c // 4, c % 4
        out[b, q * TQ : (q + 1) * TQ, :] = res.results[c]["out"]
    return out


# revision 23
# speedup vs baseline: 1.1046x; 1.0561x over previous
"""Trainium2 Bass kernel for MultiHeadSelfAttention + residual + LayerNorm.

Problem: x[2, 2048, 1024], 16 heads, head_dim 64, fp32 I/O.
  Q/K/V = x @ W{q,k,v}.T + b;  attn = softmax(Q K^T / 8) V
  out = attn-concat @ Wo.T + bo;  y = LayerNorm(x + out)

Sharding (8 cores, collective-free):
  core c: batch b = c // 4, query-token strip q = c % 4 (512 tokens).
  Each core computes K/V for its whole batch (all 16 heads), Q for its
  512 query tokens, full attention + out-proj + LayerNorm for them, and
  outputs out[512, 1024].  K/V projection is recomputed 4x per batch --
  cheaper than the measured collective alternatives for this shape.

v2 design (PE was the bottleneck at ~259us busy of a 310us span):
  - ALL matmuls run fp8 e4m3 (x, Wq, Wk, Wv, Wo, K, Q, P, V, ctx in fp8)
    with DoubleRow pair-contraction for the projections and out-proj:
    halves the MM count of K/Q/out projections vs the bf16 baseline.
  - Weights are pre-scaled host-side to sit in the e4m3 normal range
    (Wq,Wk x8; Wv x16; Wo x32) and the residual input is pre-scaled x512
    so the out-proj PSUM lands at 512*(out+x).  LayerNorm is
    scale-invariant, so only eps is scaled (x512^2); gamma/beta epilogue
    is unaffected.  Score scale absorbs the 8*8: exp(scale=0.125/64).
  - Scores (contraction = head_dim 64 -> only half the PE rows) run as
    row-tiled CONCURRENT pairs: head-even weights in array rows 0-63,
    head-odd in rows 64-127 (tile_position auto-derived from the base
    partition), sharing the 128x128 array per key chunk.
  - x is loaded once (fp8, 2MB) and stays resident; the bf16 copy of x
    is gone entirely (halves input DMA).
  - softmax exp: most chunks on the Scalar engine (Exp LUT, fp8 out);
    every 4th chunk is computed on the Vector engine instead with a
    Schraudolph bit-trick: bits = round(a*logits + 55.54) as uint8,
    bitcast to e4m3 (DVE converts with round-to-nearest, saturating).
    Zero-bias constant so ACT and DVE chunks agree in expectation;
    softmax renormalizes the shared multiplicative bias away anyway
    (the ones-column in V gives rowsums of the SAME quantized P).
  - rowsum reciprocal runs directly on the PSUM rowsum row ([1,512]),
    then gpsimd partition-broadcasts the reciprocal (drops one DVE
    copy per head vs broadcasting the raw sum).
  - K/Q j-tiles j2..j7 and V quads 1..3 are emitted as PE filler inside
    the attention windows (two j-tiles ahead, half a quad per pair) so
    the PE never idles and the HAM clock stays warm.
Measured end-to-end Frobenius rel err ~1e-3 (tolerance 2e-2; errors in
the attention path are suppressed ~100x by the residual).
"""

import numpy as np
import ml_dtypes

P = 128
D = 1024
S = 2048
B = 2
H = 16
DH = 64
TQ = 512  # query tokens per core
N_CORES = 8

F32 = np.float32
BF16 = ml_dtypes.bfloat16
F8 = ml_dtypes.float8_e4m3fn

# host-side pre-scales (see docstring)
WQ_S = 8.0
WK_S = 8.0
WV_S = 16.0
WO_S = 32.0
RES_S = WV_S * WO_S  # 512
LN_EPS = 1e-5 * RES_S * RES_S
SC_SCALE = 0.125 / (WQ_S * WK_S)  # exp scale on raw psum logits
A_SCHR = SC_SCALE * 8.0 / 0.6931471805599453
B_SCHR = 55.54
SCHR = (2, 6, 10, 14)  # chunk indices (of 16 per head-pair) done on DVE

_BUILT = {}

import os

KPHASE = int(os.environ.get("KPHASE", "3"))
KDEBUG = int(os.environ.get("KDEBUG", "0"))


def _build_nc(apply_gb=True):
    from contextlib import ExitStack

    import concourse.tile as tile
    from concourse import bacc, mybir

    bf = mybir.dt.bfloat16
    f8 = mybir.dt.float8e4
    u8 = mybir.dt.uint8
    f32 = mybir.dt.float32
    AX = mybir.AxisListType.X
    OP = mybir.AluOpType
    AF = mybir.ActivationFunctionType
    DR = mybir.MatmulPerfMode.DoubleRow

    nc = bacc.Bacc(
        "TRN2",
        target_bir_lowering=False,
        debug=False,
        enable_asserts=False,
        num_devices=N_CORES,
    )

    # ---- DRAM I/O ----
    x8_d = nc.dram_tensor("x8", [8, P, S], f8, kind="ExternalInput").ap()
    wq_d = nc.dram_tensor("wq", [D, D], f8, kind="ExternalInput").ap()
    wk_d = nc.dram_tensor("wk", [D, D], f8, kind="ExternalInput").ap()
    wv_d = nc.dram_tensor("wv", [D, D], f8, kind="ExternalInput").ap()
    wo_d = nc.dram_tensor("wo", [D, D], f8, kind="ExternalInput").ap()
    qb_d = nc.dram_tensor("qb", [P, 8], f32, kind="ExternalInput").ap()
    kb_d = nc.dram_tensor("kb", [P, 8], f32, kind="ExternalInput").ap()
    # rows: [16*bv | gamma | beta]
    rows_d = nc.dram_tensor("rows", [1, 3 * D], bf, kind="ExternalInput").ap()
    xres_d = nc.dram_tensor("xres", [TQ, D], f32, kind="ExternalInput").ap()
    out_d = nc.dram_tensor("out", [TQ, D], f32, kind="ExternalOutput").ap()
    if KDEBUG:
        dbg_k = nc.dram_tensor("dbg_k", [P, 8 * S], u8, kind="ExternalOutput").ap()
        dbg_q = nc.dram_tensor("dbg_q", [P, 8 * TQ], u8, kind="ExternalOutput").ap()
        dbg_v = nc.dram_tensor("dbg_v", [P, 16 * H * (DH + 1)], u8, kind="ExternalOutput").ap()
        dbg_c = nc.dram_tensor("dbg_c", [P, 8 * TQ], u8, kind="ExternalOutput").ap()
        dbg_pt = nc.dram_tensor("dbg_pt", [P, 8, 2, 2, 512], u8, kind="ExternalOutput").ap()
        dbg_cp = nc.dram_tensor("dbg_cp", [P, 2, 512], f32, kind="ExternalOutput").ap()

    wq_t = wq_d.rearrange("(o p) n -> p o n", p=P)  # [128, 8, 1024]
    wk_t = wk_d.rearrange("(o p) n -> p o n", p=P)
    wv_t = wv_d.rearrange("(o p) n -> p o n", p=P)
    wo_t = wo_d.rearrange("(o p) n -> p o n", p=P)

    with tile.TileContext(nc) as tc:
        with ExitStack() as ctx:
            # ---- pools ----
            consts = ctx.enter_context(tc.tile_pool(name="consts", bufs=1))
            wpool = ctx.enter_context(tc.tile_pool(name="wpool", bufs=1))
            big = ctx.enter_context(tc.tile_pool(name="big", bufs=1))
            ppool = ctx.enter_context(tc.tile_pool(name="ppool", bufs=3))
            spool = ctx.enter_context(tc.tile_pool(name="spool", bufs=4))
            hpool = ctx.enter_context(tc.tile_pool(name="hpool", bufs=3))
            xrpool = ctx.enter_context(tc.tile_pool(name="xrpool", bufs=4))
            pmm = ctx.enter_context(tc.tile_pool(name="pmm", bufs=2, space="PSUM"))
            smm = ctx.enter_context(tc.tile_pool(name="smm", bufs=2, space="PSUM"))
            ctxp = ctx.enter_context(tc.tile_pool(name="ctxp", bufs=2, space="PSUM"))

            # ---- constants ----
            zero_c = consts.tile([P, 1], f32, tag="zero_c")
            nc.vector.memset(zero_c[:], 0.0)
            nc.const_aps.aps[(f32, 0.0)] = zero_c[:]
            eps_c = consts.tile([P, 1], f32, tag="eps_c")
            nc.vector.memset(eps_c[:], LN_EPS)
            ones_l = consts.tile([1, P], bf, tag="ones_l")  # matmul lhsT ones
            nc.vector.memset(ones_l[:], 1.0)
            rows_sb = consts.tile([1, 3 * D], bf, tag="rows")
            nc.sync.dma_start(rows_sb[:], rows_d[:])
            qb_sb = consts.tile([P, 8], f32, tag="qb")
            nc.sync.dma_start(qb_sb[:], qb_d[:])
            kb_sb = consts.tile([P, 8], f32, tag="kb")
            nc.sync.dma_start(kb_sb[:], kb_d[:])

            # ---- resident inputs ----
            x8_sb = wpool.tile([P, 8, S], f8, tag="x8")
            wk8_sb = wpool.tile([P, 8, D], f8, tag="wk8")
            wv8_sb = wpool.tile([P, 8, D], f8, tag="wv8")
            wq8_sb = wpool.tile([P, 8, D], f8, tag="wq8")
            wo8_sb = wpool.tile([P, 8, D], f8, tag="wo8")
            for k in range(8):
                nc.sync.dma_start(x8_sb[:, k], x8_d[k])
                nc.sync.dma_start(wk8_sb[:, k], wk_t[:, k])
            for k in range(8):
                nc.sync.dma_start(wq8_sb[:, k], wq_t[:, k])
            for k in range(8):
                nc.sync.dma_start(wv8_sb[:, k], wv_t[:, k])

            # broadcast [1, 1024] rows across partitions via rank-1 matmuls
            bv_bc = consts.tile([P, D], bf, tag="bv_bc")
            bcasts = [bv_bc]
            if apply_gb:
                ga_bc = consts.tile([P, D], bf, tag="ga_bc")
                be_bc = consts.tile([P, D], bf, tag="be_bc")
                bcasts += [ga_bc, be_bc]
            for idx, dst in enumerate(bcasts):
                for half in range(2):
                    ps = smm.tile([P, 2, 512], f32, tag="smm")
                    nc.tensor.matmul(
                        ps[:, 0],
                        ones_l[:],
                        rows_sb[:, idx * D + half * 512 : idx * D + (half + 1) * 512],
                        start=True,
                        stop=True,
                    )
                    nc.scalar.copy(dst[:, half * 512 : (half + 1) * 512], ps[:, 0])

            # ---- big activations ----
            kT8 = big.tile([P, 8, S], f8, tag="kT")  # K^T: [dh-pair part, j, token]
            qT8 = big.tile([P, 8, TQ], f8, tag="qT")
            # V' per (tk-chunk, head): [128 tok, 65] (64 dh + ones col)
            v_sb = big.tile([P, 16, H, DH + 1], f8, tag="v")
            nc.vector.memset(v_sb[:, :, :, DH : DH + 1], 1.0)
            ctxf = [
                big.tile([P, 2, TQ], f8, tag=f"ctxf{q}", name=f"ctxf{q}")
                for q in range(4)
            ]

            # ---- fp8 DoubleRow K/Q projection for one j-tile (all strips) ----
            def kq_proj_j(j):
                steps = []
                for s in range(4):
                    def kstep(s=s, j=j):
                        ps = pmm.tile([P, 512], f32, tag="pmm")
                        for c2 in range(4):
                            nc.tensor.matmul(
                                ps[:],
                                wk8_sb[:, 2 * c2 : 2 * c2 + 2, j * P : (j + 1) * P],
                                x8_sb[:, 2 * c2 : 2 * c2 + 2, s * 512 : (s + 1) * 512],
                                start=(c2 == 0),
                                stop=(c2 == 3),
                                perf_mode=DR,
                            )
                        nc.vector.tensor_scalar_add(
                            kT8[:, j, s * 512 : (s + 1) * 512], ps[:], kb_sb[:, j : j + 1]
                        )
                    steps.append(kstep)

                def qstep(j=j):
                    ps = pmm.tile([P, 512], f32, tag="pmm")
                    for c2 in range(4):
                        nc.tensor.matmul(
                            ps[:],
                            wq8_sb[:, 2 * c2 : 2 * c2 + 2, j * P : (j + 1) * P],
                            x8_sb[:, 2 * c2 : 2 * c2 + 2, 0:512],
                            start=(c2 == 0),
                            stop=(c2 == 3),
                            perf_mode=DR,
                        )
                    nc.vector.tensor_scalar_add(qT8[:, j], ps[:], qb_sb[:, j : j + 1])
                steps.append(qstep)
                return steps

            # ---- fp8 DoubleRow V projection steps for one quad ----
            def v_proj_quad(quad):
                steps = []
                for s in range(4):
                    for tc_ in range(4):
                        def vstep(s=s, tc_=tc_, quad=quad):
                            tchunk = s * 4 + tc_
                            ps = pmm.tile([P, 512], f32, tag="pmm")
                            for c2 in range(4):
                                nc.tensor.matmul(
                                    ps[:, : 4 * DH],
                                    x8_sb[:, 2 * c2 : 2 * c2 + 2, tchunk * P : (tchunk + 1) * P],
                                    wv8_sb[:, 2 * c2 : 2 * c2 + 2, quad * 256 : (quad + 1) * 256],
                                    start=(c2 == 0),
                                    stop=(c2 == 3),
                                    perf_mode=DR,
                                )
                            nc.vector.tensor_tensor(
                                v_sb[:, tchunk, quad * 4 : (quad + 1) * 4, 0:DH],
                                ps[:, : 4 * DH].rearrange("p (h d) -> p h d", d=DH),
                                bv_bc[:, quad * 256 : (quad + 1) * 256].rearrange(
                                    "p (h d) -> p h d", d=DH
                                ),
                                OP.add,
                            )
                        steps.append(vstep)
                return steps

            # upfront: j0, j1 K/Q projections only; V quad 0 streams in as
            # pair-0 filler so the first exp can start ~15us earlier
            for st in kq_proj_j(0):
                st()
            for st in kq_proj_j(1):
                st()

            # filler schedule per attention pair (consumed one per chunk)
            def dma_fill():
                for k in range(8):
                    nc.sync.dma_start(wo8_sb[:, k], wo_t[:, k])

            xrs = []

            def xres_fill():
                for tt in range(4):
                    xr = xrpool.tile([P, D], f32, tag="xr", name=f"xr{tt}")
                    nc.sync.dma_start(xr[:], xres_d[tt * P : (tt + 1) * P, :])
                    xrs.append(xr)

            vq = [v_proj_quad(q) for q in range(4)]
            fillers = [
                vq[0],
                kq_proj_j(2) + kq_proj_j(3) + vq[1][:6],
                vq[1][6:] + kq_proj_j(4),
                vq[2],
                kq_proj_j(5) + vq[3][:8],
                vq[3][8:] + kq_proj_j(6),
                kq_proj_j(7) + [dma_fill, xres_fill],
                [],
            ]

            # ---- attention: 8 head-pairs, row-tiled concurrent score MMs ----
            for pr in range(8):
                he, ho = 2 * pr, 2 * pr + 1
                fill = list(fillers[pr])
                if KPHASE < 2:
                    for st in fill:
                        st()
                    continue
                cps_e = ctxp.tile([P, 512], f32, tag="ctx")
                cps_o = ctxp.tile([P, 512], f32, tag="ctx")
                for cc in range(8):
                    pt = ppool.tile([P, 2, 2, 512], f8, tag="pt")
                    for par in range(2):
                        c = 2 * cc + par
                        sc = smm.tile([P, 2, 512], f32, tag="smm")
                        nc.tensor.matmul(
                            sc[:, 0],
                            kT8[0:DH, pr, c * P : (c + 1) * P],
                            qT8[0:DH, pr],
                            start=True,
                            stop=True,
                        )
                        nc.tensor.matmul(
                            sc[:, 1],
                            kT8[DH:P, pr, c * P : (c + 1) * P],
                            qT8[DH:P, pr],
                            start=True,
                            stop=True,
                        )
                        if c in SCHR:
                            nc.vector.tensor_scalar(
                                pt[:, par].bitcast(u8),
                                sc[:],
                                A_SCHR,
                                B_SCHR,
                                OP.mult,
                                OP.add,
                            )
                        else:
                            nc.scalar.activation(pt[:, par], sc[:], AF.Exp, scale=SC_SCALE)
                        if fill:
                            fill.pop(0)()
                    nc.tensor.matmul(
                        cps_e[: DH + 1],
                        v_sb[:, 2 * cc : 2 * cc + 2, he],
                        pt[:, :, 0],
                        start=(cc == 0),
                        stop=(cc == 7),
                        perf_mode=DR,
                    )
                    nc.tensor.matmul(
                        cps_o[: DH + 1],
                        v_sb[:, 2 * cc : 2 * cc + 2, ho],
                        pt[:, :, 1],
                        start=(cc == 0),
                        stop=(cc == 7),
                        perf_mode=DR,
                    )
                    if KDEBUG and pr == 0:
                        nc.sync.dma_start(dbg_pt[:, cc], pt[:].bitcast(u8))
                if KDEBUG and pr == 0:
                    cpcopy = hpool.tile([P, 2, 512], f32, tag="cpdbg")
                    nc.vector.tensor_copy(cpcopy[:, 0], cps_e[:])
                    nc.vector.tensor_copy(cpcopy[:, 1], cps_o[:])
                    nc.sync.dma_start(dbg_cp[:], cpcopy[:])
                for st in fill:
                    st()
                # evacuate ctx PSUM to SBUF right away (frees the bank for the
                # next pair), then normalize from the SBUF copy
                for h, cps in ((he, cps_e), (ho, cps_o)):
                    rs = spool.tile([1, 512], f32, tag="rs")
                    nc.vector.tensor_copy(rs[:], cps[DH : DH + 1, :])
                    ri = spool.tile([1, 512], f32, tag="ri")
                    nc.vector.reciprocal_approx_fast(ri[:], rs[:])
                    rb = spool.tile([DH, 512], f32, tag="rb")
                    nc.gpsimd.partition_broadcast(rb[:], ri[:])
                    po = (h % 2) * DH
                    nc.vector.tensor_tensor(
                        ctxf[h // 4][po : po + DH, (h % 4) // 2],
                        cps[:DH],
                        rb[:],
                        OP.mult,
                    )

            # ---- out projection (fp8 DR) + residual + LayerNorm ----
            for tt in range(4):
                if KPHASE < 2:
                    continue
                xr = xrs[tt]
                if KPHASE < 3:
                    nc.sync.dma_start(out_d[tt * P : (tt + 1) * P, :], xr[:])
                    continue
                h_sb = hpool.tile([P, D], f32, tag="h")
                for half in range(2):
                    if half == 0:
                        ps = pmm.tile([P, 512], f32, tag="pmm")
                    else:
                        ps2 = smm.tile([P, 2, 512], f32, tag="smm")
                        ps = ps2[:, 0]
                    for j in range(4):
                        nc.tensor.matmul(
                            ps[:],
                            ctxf[j][:, :, tt * P : (tt + 1) * P],
                            wo8_sb[:, 2 * j : 2 * j + 2, half * 512 : (half + 1) * 512],
                            start=(j == 0),
                            stop=(j == 3),
                            perf_mode=DR,
                        )
                    # residual (+bo folded into xres host-side, x512 scale)
                    nc.vector.tensor_tensor(
                        h_sb[:, half * 512 : (half + 1) * 512],
                        ps[:],
                        xr[:, half * 512 : (half + 1) * 512],
                        OP.add,
                    )
                if KPHASE == 4:
                    nc.sync.dma_start(out_d[tt * P : (tt + 1) * P, :], h_sb[:])
                    continue
                # LayerNorm over the free dim (scale-invariant; eps pre-scaled)
                s1 = spool.tile([P, 1], f32, tag="s1")
                nc.vector.reduce_sum(s1[:], h_sb[:], axis=AX)
                y = hpool.tile([P, D], f32, tag="y")
                s2 = spool.tile([P, 1], f32, tag="s2")
                nc.scalar.activation(y[:], h_sb[:], AF.Square, accum_out=s2[:])
                mu = spool.tile([P, 1], f32, tag="mu")
                nc.scalar.mul(mu[:], s1[:], 1.0 / D)
                m2 = spool.tile([P, 1], f32, tag="m2")
                nc.scalar.square(m2[:], mu[:])
                var = spool.tile([P, 1], f32, tag="var")
                nc.vector.tensor_scalar(
                    var[:], s2[:], 1.0 / D, m2[:], OP.mult, OP.subtract
                )
                sd = spool.tile([P, 1], f32, tag="sd")
                nc.scalar.activation(sd[:], var[:], AF.Sqrt, bias=eps_c[:], scale=1.0)
                rstd = spool.tile([P, 1], f32, tag="rstd")
                nc.vector.reciprocal(rstd[:], sd[:])
                nc.vector.tensor_scalar(
                    y[:], h_sb[:], mu[:], rstd[:], OP.subtract, OP.mult
                )
                if apply_gb:
                    nc.vector.tensor_tensor(y[:], y[:], ga_bc[:], OP.mult)
                    nc.vector.tensor_tensor(y[:], y[:], be_bc[:], OP.add)
                nc.sync.dma_start(out_d[tt * P : (tt + 1) * P, :512], y[:, :512])
                nc.sync.dma_start(out_d[tt * P : (tt + 1) * P, 512:], y[:, 512:])

            if KDEBUG:
                nc.sync.dma_start(dbg_k[:], kT8[:].bitcast(u8).rearrange("p a b -> p (a b)"))
                nc.sync.dma_start(dbg_q[:], qT8[:].bitcast(u8).rearrange("p a b -> p (a b)"))
                nc.sync.dma_start(dbg_v[:], v_sb[:].bitcast(u8).rearrange("p a b c -> p (a b c)"))
                for q_ in range(4):
                    nc.sync.dma_start(
                        dbg_c[:, q_ * 2 * TQ : (q_ + 1) * 2 * TQ],
                        ctxf[q_][:].bitcast(u8).rearrange("p a b -> p (a b)"),
                    )

    nc.compile()
    return nc


def _get_nc(apply_gb=True):
    key = ("nc", apply_gb)
    if key not in _BUILT:
        _BUILT[key] = _build_nc(apply_gb)
    return _BUILT[key]


def _prep_in_maps(x, Wq, bq, Wk, bk, Wv, bv, Wo, bo, gamma, beta):
    x = np.asarray(x, F32)
    wq = np.ascontiguousarray(WQ_S * np.asarray(Wq, F32).T).astype(F8)
    wk = np.ascontiguousarray(WK_S * np.asarray(Wk, F32).T).astype(F8)
    wv = np.ascontiguousarray(WV_S * np.asarray(Wv, F32).T).astype(F8)
    wo = np.ascontiguousarray(WO_S * np.asarray(Wo, F32).T).astype(F8)
    qb = np.ascontiguousarray(WQ_S * np.asarray(bq, F32).reshape(8, P).T)
    kb = np.ascontiguousarray(WK_S * np.asarray(bk, F32).reshape(8, P).T)
    rows = (
        np.concatenate(
            [WV_S * np.asarray(bv, F32), np.asarray(gamma, F32), np.asarray(beta, F32)]
        )
        .reshape(1, 3 * D)
        .astype(BF16)
    )
    bo = np.asarray(bo, F32)
    xT = [np.ascontiguousarray(x[b].T) for b in range(B)]

    in_maps = []
    for c in range(N_CORES):
        b, q = c // 4, c % 4
        # permute: own query strip first; key order is irrelevant to attention
        perm = np.r_[q * TQ : (q + 1) * TQ, 0 : q * TQ, (q + 1) * TQ : S]
        in_maps.append(
            {
                "x8": np.ascontiguousarray(
                    xT[b][:, perm].reshape(8, P, S)
                ).astype(F8),
                "wq": wq,
                "wk": wk,
                "wv": wv,
                "wo": wo,
                "qb": qb,
                "kb": kb,
                "rows": rows,
                "xres": RES_S
                * (np.ascontiguousarray(x[b, q * TQ : (q + 1) * TQ, :]) + bo[None, :]),
            }
        )
    return in_maps


def kernel(x, Wq, bq, Wk, bk, Wv, bv, Wo, bo, gamma, beta):
    from concourse.bass_utils import run_bass_kernel_spmd

    apply_gb = not (
        np.all(np.asarray(gamma, F32) == 1.0) and np.all(np.asarray(beta, F32) == 0.0)
    )
    nc = _get_nc(apply_gb)
    in_maps = _prep_in_maps(x, Wq, bq, Wk, bk, Wv, bv, Wo, bo, gamma, beta)
    res = run_bass_kernel_spmd(nc, in_maps, core_ids=list(range(N_CORES)))
    out = np.empty((B, S, D), F32)
    for c in range(N_CORES):
        b, q = c // 4, c % 4
        out[b, q * TQ : (q + 1) * TQ, :] = res.results[c]["out"]
    return out
